# Initial kernel scaffold
#
"""FP8GroupedExperts TRN2 kernel — expert-parallel, 1-level Strassen both phases.

Per expert e (tokens pre-sorted, 2048 each):
    h   = silu(x_e @ w1[e].T) * (x_e @ w3[e].T)      # (T, HID) SwiGLU
    out = h @ w2[e].T                                 # (T, DIM)

Phase 1 (yT = W @ xT, HID on partitions), 1-level Strassen with both operand
combo sets precomputed on host; M products in PSUM, DVE assembles quadrants,
fused silu*mult, h tiles spilled to a DRAM scratch tile (hT layout).

Phase 2 (out = h @ W2^T), 1-level Strassen: w2 combos on host; h combos built
on-chip from the spilled hT strips; M products accumulate in PSUM over the
2816-deep contraction; C-assembly accumulates +-M_i into f32 acc tiles (DVE,
one PSUM operand per op) which DMA straight to the output.

Strassen maps (classic):
  A-combos: (A11+A22, A21+A22, A11, A22, A11+A12, A21-A11, A12-A22)
  B-combos: (B11+B22, B11, B12-B22, B21-B11, B22, B11+B12, B21+B22)
  C11 = M1+M4-M5+M7 ; C12 = M3+M5 ; C21 = M2+M4 ; C22 = M1-M2+M3+M6
"""

import sys

for _p in ("/opt/trn_rl_repo",):
    if _p not in sys.path:
        sys.path.append(_p)

import numpy as np
import ml_dtypes

import concourse.bacc as bacc
import concourse.mybir as mybir
import concourse.tile as tile
from concourse.bass_utils import run_bass_kernel_spmd

E = 8
DIM = 2048
HID = 5632
T = 2048
P = 128

KH = DIM // 2     # 1024 contraction half (phase 1)
HH = HID // 2     # 2816 h half
TH = T // 2       # 1024 token half
KSUB = KH // P    # 8 k-subtiles per product (phase 1)
RSUB = HH // P    # 22 h-subtiles per quadrant
NI = 7
NIW = 2 * NI      # stacked w1|w3 combos
KG = 2            # k-subtiles per weight chunk
NKG = KSUB // KG  # 4
HSUB = HID // P   # 44

DC = 512          # phase-2 output column chunk
NDC = KH // DC    # 2 chunks per D-half
W2G = 2           # h-subtiles per w2c chunk DMA
NTSL = 4          # 128-token subtiles per 512-token row chunk

BF16 = mybir.dt.bfloat16
F32 = mybir.dt.float32
ADD = mybir.AluOpType.add
SUB = mybir.AluOpType.subtract
MULT = mybir.AluOpType.mult

# per-i contributions of M_i to acc quadrants (q, sign): q: 0=C11,1=C12,2=C21,3=C22
ACC_MAP = {
    0: ((0, +1), (3, +1)),
    1: ((2, +1), (3, -1)),
    2: ((1, +1), (3, +1)),
    3: ((0, +1), (2, +1)),
    4: ((0, -1), (1, +1)),
    5: ((3, +1),),
    6: ((0, +1),),
}
# process plain-operand products first so on-chip combo DVE work hides under
# the PE; all first contributions are positive (copy-init)
I_ORDER = (2, 3, 0, 1, 4, 5, 6)

_BUILD_CACHE = {}


def _build():
    nc = bacc.Bacc(None, target_bir_lowering=False)

    w13c = nc.dram_tensor("w13c", [KH, HH, NIW], BF16, kind="ExternalInput")
    xc = nc.dram_tensor("xc", [NI, KH, TH], BF16, kind="ExternalInput")
    w2c = nc.dram_tensor("w2c", [HH, NI, KH], BF16, kind="ExternalInput")
    out = nc.dram_tensor("out", [T, DIM], F32, kind="ExternalOutput")

    w13_r = w13c.rearrange("(ko p) h i -> p ko h i", p=P)   # [128, 8, 2816, 14]
    xc_r = xc.rearrange("i (ko p) t -> p i ko t", p=P)      # [128, 7, 8, 1024]
    w2c_r = w2c.rearrange("(hb p) i d -> p hb i d", p=P)    # [128, 22, 7, 1024]

    with tile.TileContext(nc) as tc:
        with tc.tile_pool(name="dram", bufs=1, space="DRAM") as dram:
            # h spill, hT layout [128, 44 h-subtiles, 2048 tokens]
            hsp = dram.tile([P, HSUB, T], BF16, name="hsp")

            # ================= phase 1 =================
            with (
                tc.tile_pool(name="xp", bufs=1) as xp,
                tc.tile_pool(name="wp", bufs=8) as wp,
                tc.tile_pool(name="tp", bufs=1) as tp,
                tc.tile_pool(name="oph", bufs=6) as oph,
                tc.tile_pool(name="ps", bufs=8, space="PSUM") as ps,
            ):
                def load_wch(r, kg, name=None):
                    w13 = wp.tile([P, KG, P, NIW], BF16, tag="wch", name=name)
                    nc.sync.dma_start(
                        w13[:],
                        w13_r[:, kg * KG:(kg + 1) * KG, r * P:(r + 1) * P, :],
                    )
                    return w13

                def assemble(pm, side):
                    """7 M psum tiles -> 4 y quadrant tiles (bf16 sbuf)."""
                    c0 = tp.tile([P, 512], BF16, tag="c0", name="c0")
                    c2 = tp.tile([P, 512], BF16, tag="c2", name="c2")
                    c3 = tp.tile([P, 512], BF16, tag="c3", name="c3")
                    c4 = tp.tile([P, 512], BF16, tag="c4", name="c4")
                    nc.vector.tensor_copy(c0[:], pm[0][:])
                    nc.vector.tensor_copy(c2[:], pm[2][:])
                    nc.vector.tensor_copy(c3[:], pm[3][:])
                    nc.vector.tensor_copy(c4[:], pm[4][:])
                    y = [tp.tile([P, 512], BF16, tag=f"y{side}_{q}",
                                 name=f"y{side}_{q}")
                         for q in range(4)]
                    ta = tp.tile([P, 512], BF16, tag=f"ta{side}")
                    tb = tp.tile([P, 512], BF16, tag=f"tb{side}")
                    nc.vector.tensor_tensor(y[1][:], c2[:], c4[:], ADD)
                    nc.vector.tensor_tensor(y[2][:], pm[1][:], c3[:], ADD)
                    nc.vector.tensor_tensor(ta[:], c0[:], c3[:], ADD)
                    nc.vector.tensor_tensor(tb[:], pm[6][:], c4[:], SUB)
                    nc.vector.tensor_tensor(y[0][:], ta[:], tb[:], ADD)
                    nc.vector.tensor_tensor(ta[:], pm[1][:], c0[:], SUB)
                    nc.vector.tensor_tensor(tb[:], pm[5][:], c2[:], ADD)
                    nc.vector.scalar_tensor_tensor(
                        y[3][:], ta[:], -1.0, tb[:], MULT, ADD)
                    return y

                # HAM pre-warm while the first DMAs land
                wz = tp.tile([P, 512], BF16, tag="wz", bufs=1)
                nc.gpsimd.memset(wz[:], 0.0)
                wps = ps.tile([P, 512], F32, tag="pm", name="warm_ps")
                for _ in range(24):
                    nc.tensor.matmul(wps[:], wz[:, 0:P], wz[:],
                                     start=True, stop=True)

                # startup: interleave wch/xcs DMAs in consumption order
                xcs = xp.tile([P, NI, KSUB, TH], BF16, tag="xcs", name="xcs")
                wch0 = []
                for kg in range(NKG):
                    wch0.append(load_wch(0, kg, f"wch_0_{kg}"))
                    for i in range(NI):
                        nc.sync.dma_start(
                            xcs[:, i, kg * KG:(kg + 1) * KG, 0:512],
                            xc_r[:, i, kg * KG:(kg + 1) * KG, 0:512],
                        )
                for i in range(NI):
                    nc.sync.dma_start(
                        xcs[:, i, :, 512:TH],
                        xc_r[:, i, :, 512:TH],
                    )
                pending_wch = wch0

                for r in range(RSUB):
                    wch = pending_wch
                    pending_wch = None
                    for cc in range(2):     # 512-col chunk within B-combos
                        c0c = cc * 512
                        for side in range(2):   # 0: w1, 1: w3
                            pm = [ps.tile([P, 512], F32, tag="pm",
                                          name=f"pm{side}_{i}")
                                  for i in range(NI)]
                            for kg in range(NKG):
                                for i in range(NI):
                                    for kl in range(KG):
                                        k = kg * KG + kl
                                        nc.tensor.matmul(
                                            pm[i][:],
                                            wch[kg][:, kl, :, i + side * NI],
                                            xcs[:, i, k, c0c:c0c + 512],
                                            start=(k == 0),
                                            stop=(k == KSUB - 1),
                                        )
                            if side == 0:
                                y1 = assemble(pm, 1)
                            else:
                                y3 = assemble(pm, 3)
                        if cc == 0 and r + 1 < RSUB:
                            pending_wch = [load_wch(r + 1, kg)
                                           for kg in range(NKG)]
                        for q in range(4):
                            hq, tq = q >> 1, q & 1
                            smp = tp.tile([P, 512], BF16, tag=f"smp_{q}")
                            nc.scalar.activation(
                                smp[:], y1[q][:],
                                mybir.ActivationFunctionType.Silu)
                            ht = oph.tile([P, 512], BF16, tag="ht")
                            nc.vector.tensor_tensor(ht[:], smp[:], y3[q][:],
                                                    MULT)
                            nc.sync.dma_start(
                                hsp[:, hq * RSUB + r,
                                    tq * TH + c0c:tq * TH + c0c + 512],
                                ht[:],
                            )

            # ================= phase 2 =================
            with (
                tc.tile_pool(name="hp2", bufs=1) as hp2,
                tc.tile_pool(name="cp2", bufs=1) as cp2,
                tc.tile_pool(name="rp2", bufs=4) as rp2,
                tc.tile_pool(name="ap2", bufs=1) as ap2,
                tc.tile_pool(name="w2p", bufs=3) as w2p,
                tc.tile_pool(name="ps2", bufs=8, space="PSUM") as ps2,
            ):
                for rc in range(2):      # 512-token row chunk of each T-half
                    tc0 = rc * 512
                    # strips (hT layout): A11 = h[T0, H0], A22 = h[T1, H1]
                    a11s = hp2.tile([P, RSUB, 512], BF16, tag="a11",
                                    name=f"a11_{rc}")
                    a22s = hp2.tile([P, RSUB, 512], BF16, tag="a22",
                                    name=f"a22_{rc}")
                    nc.sync.dma_start(a11s[:], hsp[:, 0:RSUB, tc0:tc0 + 512])
                    nc.sync.dma_start(a22s[:],
                                      hsp[:, RSUB:HSUB,
                                          TH + tc0:TH + tc0 + 512])
                    # combo strips; A12 = h[T0, H1], A21 = h[T1, H0] streamed
                    s1 = cp2.tile([P, RSUB, 512], BF16, tag="s1",
                                  name=f"s1_{rc}")
                    s2 = cp2.tile([P, RSUB, 512], BF16, tag="s2",
                                  name=f"s2_{rc}")
                    s5 = cp2.tile([P, RSUB, 512], BF16, tag="s5",
                                  name=f"s5_{rc}")
                    s6 = cp2.tile([P, RSUB, 512], BF16, tag="s6",
                                  name=f"s6_{rc}")
                    s7 = cp2.tile([P, RSUB, 512], BF16, tag="s7",
                                  name=f"s7_{rc}")
                    nc.vector.tensor_tensor(s1[:], a11s[:], a22s[:], ADD)
                    for hb in range(RSUB):
                        a12t = rp2.tile([P, 512], BF16, tag="a12t")
                        nc.sync.dma_start(
                            a12t[:],
                            hsp[:, RSUB + hb, tc0:tc0 + 512])
                        nc.vector.tensor_tensor(
                            s5[:, hb, :], a11s[:, hb, :], a12t[:], ADD)
                        nc.vector.tensor_tensor(
                            s7[:, hb, :], a12t[:], a22s[:, hb, :], SUB)
                        a21t = rp2.tile([P, 512], BF16, tag="a21t")
                        nc.sync.dma_start(
                            a21t[:],
                            hsp[:, hb, TH + tc0:TH + tc0 + 512])
                        nc.vector.tensor_tensor(
                            s2[:, hb, :], a21t[:], a22s[:, hb, :], ADD)
                        nc.vector.tensor_tensor(
                            s6[:, hb, :], a21t[:], a11s[:, hb, :], SUB)
                    amap = [s1, s2, a11s, a22s, s5, s6, s7]

                    for dc in range(NDC):
                        d0 = dc * DC
                        acc = [[ap2.tile([P, DC], F32, tag=f"acc_{q}_{tsl}",
                                         name=f"acc_{q}_{tsl}")
                                for tsl in range(NTSL)] for q in range(4)]
                        acc_init = set()
                        for i in I_ORDER:
                            pmt = [ps2.tile([P, DC], F32, tag="pm2",
                                            name=f"pm2_{tsl}")
                                   for tsl in range(NTSL)]
                            for hg in range(RSUB // W2G):
                                w2g = w2p.tile([P, W2G, DC], BF16, tag="w2g")
                                nc.sync.dma_start(
                                    w2g[:],
                                    w2c_r[:, hg * W2G:(hg + 1) * W2G, i,
                                          d0:d0 + DC],
                                )
                                for hl in range(W2G):
                                    hb = hg * W2G + hl
                                    for tsl in range(NTSL):
                                        nc.tensor.matmul(
                                            pmt[tsl][:],
                                            amap[i][:, hb,
                                                    tsl * P:(tsl + 1) * P],
                                            w2g[:, hl, :],
                                            start=(hb == 0),
                                            stop=(hb == RSUB - 1),
                                        )
                            for tsl in range(NTSL):
                                for q, sgn in ACC_MAP[i]:
                                    a = acc[q][tsl]
                                    if q not in acc_init:
                                        nc.vector.tensor_copy(a[:],
                                                              pmt[tsl][:])
                                    else:
                                        nc.vector.tensor_tensor(
                                            a[:], a[:], pmt[tsl][:],
                                            ADD if sgn > 0 else SUB)
                            for q, _ in ACC_MAP[i]:
                                acc_init.add(q)
                        for q in range(4):
                            tq, dq = q >> 1, q & 1
                            for tsl in range(NTSL):
                                row = tq * TH + tc0 + tsl * P
                                col = dq * KH + d0
                                nc.sync.dma_start(
                                    out[row:row + P, col:col + DC],
                                    acc[q][tsl][:],
                                )

    nc.compile()
    return nc


def _get_nc():
    if "nc" not in _BUILD_CACHE:
        _BUILD_CACHE["nc"] = _build()
    return _BUILD_CACHE["nc"]


def _a_combos(M, half0, half1):
    """classic Strassen A-side combos of a 2x2-blocked matrix."""
    A11 = M[:half0, :half1]
    A12 = M[:half0, half1:]
    A21 = M[half0:, :half1]
    A22 = M[half0:, half1:]
    return (A11 + A22, A21 + A22, A11, A22, A11 + A12, A21 - A11, A12 - A22)


def _b_combos(M, half0, half1):
    B11 = M[:half0, :half1]
    B12 = M[:half0, half1:]
    B21 = M[half0:, :half1]
    B22 = M[half0:, half1:]
    return (B11 + B22, B11, B12 - B22, B21 - B11, B22, B11 + B12, B21 + B22)


def _prep_inputs(x, num_tokens_per_expert, w1, w2, w3):
    x = np.asarray(x, dtype=np.float32)
    w1 = np.asarray(w1, dtype=np.float32)
    w2 = np.asarray(w2, dtype=np.float32)
    w3 = np.asarray(w3, dtype=np.float32)
    counts = np.asarray(num_tokens_per_expert).astype(np.int64)
    offs = np.concatenate([[0], np.cumsum(counts)])

    in_maps = []
    for e in range(E):
        n_e = int(counts[e])
        if n_e > T:
            raise ValueError(f"expert {e} has {n_e} tokens > capacity {T}")
        xe = x[offs[e]:offs[e] + n_e]
        if n_e < T:
            xe = np.concatenate(
                [xe, np.zeros((T - n_e, DIM), dtype=np.float32)], axis=0
            )
        w13 = np.empty((KH, HH, NIW), dtype=np.float32)
        for i, m in enumerate(_a_combos(w1[e], HH, KH)):
            w13[:, :, i] = m.T
        for i, m in enumerate(_a_combos(w3[e], HH, KH)):
            w13[:, :, NI + i] = m.T
        xct = np.stack(_b_combos(np.ascontiguousarray(xe.T), KH, TH), axis=0)
        w2T = np.ascontiguousarray(w2[e].T)            # [HID, DIM]
        w2cc = np.stack(_b_combos(w2T, HH, KH), axis=1)  # [2816, 7, 1024]
        in_maps.append({
            "w13c": w13.astype(ml_dtypes.bfloat16),
            "xc": xct.astype(ml_dtypes.bfloat16),
            "w2c": np.ascontiguousarray(w2cc).astype(ml_dtypes.bfloat16),
        })
    return in_maps, counts


def _run(inputs, **run_kwargs):
    in_maps, counts = _prep_inputs(
        inputs["x"], inputs["num_tokens_per_expert"],
        inputs["w1"], inputs["w2"], inputs["w3"],
    )
    nc = _get_nc()
    res = run_bass_kernel_spmd(nc, in_maps, core_ids=list(range(E)),
                               **run_kwargs)
    pieces = [res.results[e]["out"][: int(counts[e])] for e in range(E)]
    full = np.concatenate(pieces, axis=0).astype(np.float32)
    return full, res


def kernel(**inputs):
    out, _ = _run(inputs)
    return out


if __name__ == "__main__":
    rng = np.random.default_rng(0)
    ins = {
        "x": rng.standard_normal((E * T, DIM), dtype=np.float32),
        "num_tokens_per_expert": np.full((E,), T, dtype=np.int64),
        "w1": rng.standard_normal((E, HID, DIM), dtype=np.float32) * 0.02,
        "w2": rng.standard_normal((E, DIM, HID), dtype=np.float32) * 0.02,
        "w3": rng.standard_normal((E, HID, DIM), dtype=np.float32) * 0.02,
    }
    got = kernel(**ins)
    print("out shape:", got.shape, got.dtype)



# revision 1
# speedup vs baseline: 1.0238x; 1.0238x over previous
"""FP8GroupedExperts TRN2 kernel — expert-parallel, 1-level Strassen both phases.

Per expert e (tokens pre-sorted, 2048 each):
    h   = silu(x_e @ w1[e].T) * (x_e @ w3[e].T)      # (T, HID) SwiGLU
    out = h @ w2[e].T                                 # (T, DIM)

Phase 1 (yT = W @ xT, HID on partitions), 1-level Strassen with both operand
combo sets precomputed on host; M products in PSUM, DVE assembles quadrants,
fused silu*mult, h tiles spilled to a DRAM scratch tile (hT layout).

Phase 2 (out = h @ W2^T), 1-level Strassen: w2 combos on host; h combos built
on-chip from the spilled hT strips; M products accumulate in PSUM over the
2816-deep contraction; C-assembly accumulates +-M_i into f32 acc tiles (DVE,
one PSUM operand per op) which DMA straight to the output.

Strassen maps (classic):
  A-combos: (A11+A22, A21+A22, A11, A22, A11+A12, A21-A11, A12-A22)
  B-combos: (B11+B22, B11, B12-B22, B21-B11, B22, B11+B12, B21+B22)
  C11 = M1+M4-M5+M7 ; C12 = M3+M5 ; C21 = M2+M4 ; C22 = M1-M2+M3+M6
"""

import sys

for _p in ("/opt/trn_rl_repo",):
    if _p not in sys.path:
        sys.path.append(_p)

import numpy as np
import ml_dtypes

import concourse.bacc as bacc
import concourse.mybir as mybir
import concourse.tile as tile
from concourse.bass_utils import run_bass_kernel_spmd

E = 8
DIM = 2048
HID = 5632
T = 2048
P = 128

KH = DIM // 2     # 1024 contraction half (phase 1)
HH = HID // 2     # 2816 h half
TH = T // 2       # 1024 token half
KSUB = KH // P    # 8 k-subtiles per product (phase 1)
RSUB = HH // P    # 22 h-subtiles per quadrant
NI = 7
NIW = 2 * NI      # stacked w1|w3 combos
KG = 2            # k-subtiles per weight chunk
NKG = KSUB // KG  # 4
HSUB = HID // P   # 44

DC = 512          # phase-2 output column chunk
NDC = KH // DC    # 2 chunks per D-half
W2G = 2           # h-subtiles per w2c chunk DMA
NTSL = 4          # 128-token subtiles per 512-token row chunk

BF16 = mybir.dt.bfloat16
F32 = mybir.dt.float32
ADD = mybir.AluOpType.add
SUB = mybir.AluOpType.subtract
MULT = mybir.AluOpType.mult

# per-i contributions of M_i to acc quadrants (q, sign): q: 0=C11,1=C12,2=C21,3=C22
ACC_MAP = {
    0: ((0, +1), (3, +1)),
    1: ((2, +1), (3, -1)),
    2: ((1, +1), (3, +1)),
    3: ((0, +1), (2, +1)),
    4: ((0, -1), (1, +1)),
    5: ((3, +1),),
    6: ((0, +1),),
}
# process plain-operand products first so on-chip combo DVE work hides under
# the PE; all first contributions are positive (copy-init)
I_ORDER = (2, 3, 0, 1, 4, 5, 6)

_BUILD_CACHE = {}


def _build():
    nc = bacc.Bacc(None, target_bir_lowering=False)

    w13c = nc.dram_tensor("w13c", [KH, HH, NIW], BF16, kind="ExternalInput")
    xc = nc.dram_tensor("xc", [NI, KH, TH], BF16, kind="ExternalInput")
    w2c = nc.dram_tensor("w2c", [HH, NI, KH], BF16, kind="ExternalInput")
    out = nc.dram_tensor("out", [T, DIM], F32, kind="ExternalOutput")

    w13_r = w13c.rearrange("(ko p) h i -> p ko h i", p=P)   # [128, 8, 2816, 14]
    xc_r = xc.rearrange("i (ko p) t -> p i ko t", p=P)      # [128, 7, 8, 1024]
    w2c_r = w2c.rearrange("(hb p) i d -> p hb i d", p=P)    # [128, 22, 7, 1024]

    with tile.TileContext(nc) as tc:
        with tc.tile_pool(name="dram", bufs=1, space="DRAM") as dram:
            # h spill, hT layout [128, 44 h-subtiles, 2048 tokens]
            hsp = dram.tile([P, HSUB, T], BF16, name="hsp")

            # ================= phase 1 =================
            with (
                tc.tile_pool(name="xp", bufs=1) as xp,
                tc.tile_pool(name="wp", bufs=8) as wp,
                tc.tile_pool(name="tp", bufs=1) as tp,
                tc.tile_pool(name="oph", bufs=6) as oph,
                tc.tile_pool(name="ps", bufs=8, space="PSUM") as ps,
            ):
                def load_wch(r, kg, name=None):
                    w13 = wp.tile([P, KG, P, NIW], BF16, tag="wch", name=name)
                    nc.sync.dma_start(
                        w13[:],
                        w13_r[:, kg * KG:(kg + 1) * KG, r * P:(r + 1) * P, :],
                    )
                    return w13

                def assemble(pm, side):
                    """7 M psum tiles -> 4 y quadrant tiles (bf16 sbuf)."""
                    c0 = tp.tile([P, 512], BF16, tag="c0", name="c0")
                    c2 = tp.tile([P, 512], BF16, tag="c2", name="c2")
                    c3 = tp.tile([P, 512], BF16, tag="c3", name="c3")
                    c4 = tp.tile([P, 512], BF16, tag="c4", name="c4")
                    nc.vector.tensor_copy(c0[:], pm[0][:])
                    nc.vector.tensor_copy(c2[:], pm[2][:])
                    nc.vector.tensor_copy(c3[:], pm[3][:])
                    nc.vector.tensor_copy(c4[:], pm[4][:])
                    y = [tp.tile([P, 512], BF16, tag=f"y{side}_{q}",
                                 name=f"y{side}_{q}")
                         for q in range(4)]
                    ta = tp.tile([P, 512], BF16, tag=f"ta{side}")
                    tb = tp.tile([P, 512], BF16, tag=f"tb{side}")
                    nc.vector.tensor_tensor(y[1][:], c2[:], c4[:], ADD)
                    nc.vector.tensor_tensor(y[2][:], pm[1][:], c3[:], ADD)
                    nc.vector.tensor_tensor(ta[:], c0[:], c3[:], ADD)
                    nc.vector.tensor_tensor(tb[:], pm[6][:], c4[:], SUB)
                    nc.vector.tensor_tensor(y[0][:], ta[:], tb[:], ADD)
                    nc.vector.tensor_tensor(ta[:], pm[1][:], c0[:], SUB)
                    nc.vector.tensor_tensor(tb[:], pm[5][:], c2[:], ADD)
                    nc.vector.scalar_tensor_tensor(
                        y[3][:], ta[:], -1.0, tb[:], MULT, ADD)
                    return y

                # HAM pre-warm while the first DMAs land
                wz = tp.tile([P, 512], BF16, tag="wz", bufs=1)
                nc.gpsimd.memset(wz[:], 0.0)
                wps = ps.tile([P, 512], F32, tag="pm", name="warm_ps")
                for _ in range(24):
                    nc.tensor.matmul(wps[:], wz[:, 0:P], wz[:],
                                     start=True, stop=True)

                # startup: interleave wch/xcs DMAs in consumption order
                xcs = xp.tile([P, NI, KSUB, TH], BF16, tag="xcs", name="xcs")
                wch0 = []
                for kg in range(NKG):
                    wch0.append(load_wch(0, kg, f"wch_0_{kg}"))
                    for i in range(NI):
                        nc.sync.dma_start(
                            xcs[:, i, kg * KG:(kg + 1) * KG, 0:512],
                            xc_r[:, i, kg * KG:(kg + 1) * KG, 0:512],
                        )
                for i in range(NI):
                    nc.sync.dma_start(
                        xcs[:, i, :, 512:TH],
                        xc_r[:, i, :, 512:TH],
                    )
                pending_wch = wch0

                for r in range(RSUB):
                    wch = pending_wch
                    pending_wch = None
                    for cc in range(2):     # 512-col chunk within B-combos
                        c0c = cc * 512
                        for side in range(2):   # 0: w1, 1: w3
                            pm = [ps.tile([P, 512], F32, tag="pm",
                                          name=f"pm{side}_{i}")
                                  for i in range(NI)]
                            for kg in range(NKG):
                                for i in range(NI):
                                    for kl in range(KG):
                                        k = kg * KG + kl
                                        nc.tensor.matmul(
                                            pm[i][:],
                                            wch[kg][:, kl, :, i + side * NI],
                                            xcs[:, i, k, c0c:c0c + 512],
                                            start=(k == 0),
                                            stop=(k == KSUB - 1),
                                        )
                            if side == 0:
                                y1 = assemble(pm, 1)
                            else:
                                y3 = assemble(pm, 3)
                        if cc == 0 and r + 1 < RSUB:
                            pending_wch = [load_wch(r + 1, kg)
                                           for kg in range(NKG)]
                        for q in range(4):
                            hq, tq = q >> 1, q & 1
                            smp = tp.tile([P, 512], BF16, tag=f"smp_{q}")
                            nc.scalar.activation(
                                smp[:], y1[q][:],
                                mybir.ActivationFunctionType.Silu)
                            ht = oph.tile([P, 512], BF16, tag="ht")
                            nc.vector.tensor_tensor(ht[:], smp[:], y3[q][:],
                                                    MULT)
                            nc.sync.dma_start(
                                hsp[:, hq * RSUB + r,
                                    tq * TH + c0c:tq * TH + c0c + 512],
                                ht[:],
                            )

            # ================= phase 2 =================
            with (
                tc.tile_pool(name="hp2", bufs=1) as hp2,
                tc.tile_pool(name="cp2", bufs=1) as cp2,
                tc.tile_pool(name="rp2", bufs=4) as rp2,
                tc.tile_pool(name="ap2", bufs=1) as ap2,
                tc.tile_pool(name="w2p", bufs=3) as w2p,
                tc.tile_pool(name="ps2", bufs=8, space="PSUM") as ps2,
            ):
                for rc in range(2):      # 512-token row chunk of each T-half
                    tc0 = rc * 512
                    # strips (hT layout): A11 = h[T0, H0], A22 = h[T1, H1]
                    a11s = hp2.tile([P, RSUB, 512], BF16, tag="a11",
                                    name=f"a11_{rc}")
                    a22s = hp2.tile([P, RSUB, 512], BF16, tag="a22",
                                    name=f"a22_{rc}")
                    nc.sync.dma_start(a11s[:], hsp[:, 0:RSUB, tc0:tc0 + 512])
                    nc.sync.dma_start(a22s[:],
                                      hsp[:, RSUB:HSUB,
                                          TH + tc0:TH + tc0 + 512])
                    # combo strips; A12 = h[T0, H1], A21 = h[T1, H0] streamed
                    s1 = cp2.tile([P, RSUB, 512], BF16, tag="s1",
                                  name=f"s1_{rc}")
                    s2 = cp2.tile([P, RSUB, 512], BF16, tag="s2",
                                  name=f"s2_{rc}")
                    s5 = cp2.tile([P, RSUB, 512], BF16, tag="s5",
                                  name=f"s5_{rc}")
                    s6 = cp2.tile([P, RSUB, 512], BF16, tag="s6",
                                  name=f"s6_{rc}")
                    s7 = cp2.tile([P, RSUB, 512], BF16, tag="s7",
                                  name=f"s7_{rc}")
                    nc.vector.tensor_tensor(s1[:], a11s[:], a22s[:], ADD)
                    for hb in range(RSUB):
                        a12t = rp2.tile([P, 512], BF16, tag="a12t")
                        nc.sync.dma_start(
                            a12t[:],
                            hsp[:, RSUB + hb, tc0:tc0 + 512])
                        nc.vector.tensor_tensor(
                            s5[:, hb, :], a11s[:, hb, :], a12t[:], ADD)
                        nc.vector.tensor_tensor(
                            s7[:, hb, :], a12t[:], a22s[:, hb, :], SUB)
                        a21t = rp2.tile([P, 512], BF16, tag="a21t")
                        nc.sync.dma_start(
                            a21t[:],
                            hsp[:, hb, TH + tc0:TH + tc0 + 512])
                        nc.vector.tensor_tensor(
                            s2[:, hb, :], a21t[:], a22s[:, hb, :], ADD)
                        nc.vector.tensor_tensor(
                            s6[:, hb, :], a21t[:], a11s[:, hb, :], SUB)
                    amap = [s1, s2, a11s, a22s, s5, s6, s7]

                    for dc in range(NDC):
                        d0 = dc * DC
                        acc = [[ap2.tile([P, DC], F32, tag=f"acc_{q}_{tsl}",
                                         name=f"acc_{q}_{tsl}")
                                for tsl in range(NTSL)] for q in range(4)]
                        acc_init = set()
                        for i in I_ORDER:
                            pmt = [ps2.tile([P, DC], F32, tag="pm2",
                                            name=f"pm2_{tsl}")
                                   for tsl in range(NTSL)]
                            for hg in range(RSUB // W2G):
                                w2g = w2p.tile([P, W2G, DC], BF16, tag="w2g")
                                nc.sync.dma_start(
                                    w2g[:],
                                    w2c_r[:, hg * W2G:(hg + 1) * W2G, i,
                                          d0:d0 + DC],
                                )
                                for hl in range(W2G):
                                    hb = hg * W2G + hl
                                    for tsl in range(NTSL):
                                        nc.tensor.matmul(
                                            pmt[tsl][:],
                                            amap[i][:, hb,
                                                    tsl * P:(tsl + 1) * P],
                                            w2g[:, hl, :],
                                            start=(hb == 0),
                                            stop=(hb == RSUB - 1),
                                        )
                            for tsl in range(NTSL):
                                for q, sgn in ACC_MAP[i]:
                                    a = acc[q][tsl]
                                    if q not in acc_init:
                                        nc.vector.tensor_copy(a[:],
                                                              pmt[tsl][:])
                                    else:
                                        nc.vector.tensor_tensor(
                                            a[:], a[:], pmt[tsl][:],
                                            ADD if sgn > 0 else SUB)
                            for q, _ in ACC_MAP[i]:
                                acc_init.add(q)
                        for q in range(4):
                            tq, dq = q >> 1, q & 1
                            for tsl in range(NTSL):
                                row = tq * TH + tc0 + tsl * P
                                col = dq * KH + d0
                                nc.sync.dma_start(
                                    out[row:row + P, col:col + DC],
                                    acc[q][tsl][:],
                                )

    nc.compile()
    return nc


def _get_nc():
    if "nc" not in _BUILD_CACHE:
        _BUILD_CACHE["nc"] = _build()
    return _BUILD_CACHE["nc"]


def _a_combos(M, half0, half1):
    """classic Strassen A-side combos of a 2x2-blocked matrix."""
    A11 = M[:half0, :half1]
    A12 = M[:half0, half1:]
    A21 = M[half0:, :half1]
    A22 = M[half0:, half1:]
    return (A11 + A22, A21 + A22, A11, A22, A11 + A12, A21 - A11, A12 - A22)


def _b_combos(M, half0, half1):
    B11 = M[:half0, :half1]
    B12 = M[:half0, half1:]
    B21 = M[half0:, :half1]
    B22 = M[half0:, half1:]
    return (B11 + B22, B11, B12 - B22, B21 - B11, B22, B11 + B12, B21 + B22)


def _prep_inputs(x, num_tokens_per_expert, w1, w2, w3):
    x = np.asarray(x, dtype=np.float32)
    w1 = np.asarray(w1, dtype=np.float32)
    w2 = np.asarray(w2, dtype=np.float32)
    w3 = np.asarray(w3, dtype=np.float32)
    counts = np.asarray(num_tokens_per_expert).astype(np.int64)
    offs = np.concatenate([[0], np.cumsum(counts)])

    in_maps = []
    for e in range(E):
        n_e = int(counts[e])
        if n_e > T:
            raise ValueError(f"expert {e} has {n_e} tokens > capacity {T}")
        xe = x[offs[e]:offs[e] + n_e]
        if n_e < T:
            xe = np.concatenate(
                [xe, np.zeros((T - n_e, DIM), dtype=np.float32)], axis=0
            )
        w13 = np.empty((KH, HH, NIW), dtype=np.float32)
        for i, m in enumerate(_a_combos(w1[e], HH, KH)):
            w13[:, :, i] = m.T
        for i, m in enumerate(_a_combos(w3[e], HH, KH)):
            w13[:, :, NI + i] = m.T
        xct = np.stack(_b_combos(np.ascontiguousarray(xe.T), KH, TH), axis=0)
        w2T = np.ascontiguousarray(w2[e].T)            # [HID, DIM]
        w2cc = np.stack(_b_combos(w2T, HH, KH), axis=1)  # [2816, 7, 1024]
        in_maps.append({
            "w13c": w13.astype(ml_dtypes.bfloat16),
            "xc": xct.astype(ml_dtypes.bfloat16),
            "w2c": np.ascontiguousarray(w2cc).astype(ml_dtypes.bfloat16),
        })
    return in_maps, counts


def _run(inputs, **run_kwargs):
    in_maps, counts = _prep_inputs(
        inputs["x"], inputs["num_tokens_per_expert"],
        inputs["w1"], inputs["w2"], inputs["w3"],
    )
    nc = _get_nc()
    res = run_bass_kernel_spmd(nc, in_maps, core_ids=list(range(E)),
                               **run_kwargs)
    pieces = [res.results[e]["out"][: int(counts[e])] for e in range(E)]
    full = np.concatenate(pieces, axis=0).astype(np.float32)
    return full, res


def kernel(**inputs):
    out, _ = _run(inputs)
    return out


if __name__ == "__main__":
    rng = np.random.default_rng(0)
    ins = {
        "x": rng.standard_normal((E * T, DIM), dtype=np.float32),
        "num_tokens_per_expert": np.full((E,), T, dtype=np.int64),
        "w1": rng.standard_normal((E, HID, DIM), dtype=np.float32) * 0.02,
        "w2": rng.standard_normal((E, DIM, HID), dtype=np.float32) * 0.02,
        "w3": rng.standard_normal((E, HID, DIM), dtype=np.float32) * 0.02,
    }
    got = kernel(**ins)
    print("out shape:", got.shape, got.dtype)

